# revision 3
# baseline (speedup 1.0000x reference)
"""Trainium2 Bass kernel for nn_CgTransform (L=7, T=128, 8 NeuronCores).

Math: for each (l1,l2) block pair and each kept output row k=(l,m):
    OUT_k[t1,t2] = sum_{i,j} C[k, l1^2+i, l2^2+j] * clms[l1^2+i, t1] * clms[l2^2+j, t2]
                 = A1^T (C_k A2)   with A1/A2 the clms row-blocks.

Device restructuring (per output row k = one "slot"):
  stage 1:  W_k[i, t2] = sum_j C_k[i,j] * clms[l2^2+j, t2]
            -> one dense matmul per 128-partition chunk: W = S^T.T @ clms,
               where S^T packs C_k columns; 4 slots per chunk at 32-partition
               bands (contraction 64, out 128 partitions).
  stage 2:  OUT_k[t1,t2] = sum_i A1pad[i,t1] * W_k[i,t2]
            -> one [32]-contraction matmul per slot (lhsT = padded A1 block at
               band 32b, rhs = W band), tile_position=(32b, 0).
            HW constraint: matmuls with different tile_positions must not share
            a PSUM bank -> band-major grouping: each PSUM tile [128,512] takes
            4 same-band matmuls from 4 consecutive chunks.

Sharding: the 2416 global output rows (pair-major, k ascending) are split into
8 contiguous runs of 302 rows; each core computes its rows and writes
[t1, slot, t2]-layout output; the host transposes/reassembles the 8 tuples.
"""
import os
import sys
import types

if "/opt/trn_rl_repo" not in sys.path:
    sys.path.insert(0, "/opt/trn_rl_repo")

import numpy as np

L = 7
LSIZE = (L + 1) ** 2          # 64
T = 128
NCORES = 8
SLOT_P = 32                   # partitions per slot band (d1<=15, padded)
BANDS = 4                     # slots per chunk
CHUNK_COLS = 128
GROUP_CHUNKS = 4              # chunks per output group (one 2048-col staging)

# ---------------------------------------------------------------- pair table
PAIRS = []                    # (l1, l2, lo, hi, rows, row_offset)
_off = 0
for _l1 in range(L + 1):
    for _l2 in range(L + 1):
        _lo, _hi = abs(_l1 - _l2), min(_l1 + _l2, L)
        _rows = (_hi + 1) ** 2 - _lo * _lo
        PAIRS.append((_l1, _l2, _lo, _hi, _rows, _off))
        _off += _rows
TOTAL_ROWS = _off             # 2416
assert TOTAL_ROWS % NCORES == 0
SLOTS = TOTAL_ROWS // NCORES  # 302 per core
CHUNKS = -(-SLOTS // BANDS)   # 76
assert CHUNKS % GROUP_CHUNKS == 0
GROUPS = CHUNKS // GROUP_CHUNKS  # 19
SLOTS_PAD = CHUNKS * BANDS    # 304
ST_COLS = CHUNKS * CHUNK_COLS    # 9728
OUT_COLS = SLOTS_PAD * T         # 38912

# global slot -> (pair_idx, k_abs)
SLOT_TABLE = []
for _pi, (_l1, _l2, _lo, _hi, _rows, _po) in enumerate(PAIRS):
    for _k in range(_lo * _lo, (_hi + 1) ** 2):
        SLOT_TABLE.append((_pi, _k))
assert len(SLOT_TABLE) == TOTAL_ROWS


def _slot_colbase(s):
    """Column base of local slot s in the per-core output (band-major groups)."""
    c, b = divmod(s, BANDS)
    g, cc = divmod(c, GROUP_CHUNKS)
    return 2048 * g + 512 * b + 128 * cc


_NC = None


def _build_nc():
    import concourse.bacc as bacc
    import concourse.mybir as mybir
    import concourse.tile as tile

    F32 = mybir.dt.float32
    nc = bacc.Bacc("TRN2", target_bir_lowering=False, debug=False, num_devices=1)
    d_clms = nc.dram_tensor("clms", [LSIZE, T], F32, kind="ExternalInput")
    d_st = nc.dram_tensor("s_t", [LSIZE, ST_COLS], F32, kind="ExternalInput")
    d_a1 = nc.dram_tensor("a1s", [128, ST_COLS], F32, kind="ExternalInput")
    d_out = nc.dram_tensor("o", [128, OUT_COLS], F32, kind="ExternalOutput")

    with tile.TileContext(nc) as tc:
        with (
            tc.tile_pool(name="big", bufs=1) as big,
            tc.tile_pool(name="wpool", bufs=12) as wpool,
            tc.tile_pool(name="stage", bufs=3) as stage,
            tc.tile_pool(name="wps", bufs=2, space="PSUM") as wps_pool,
            tc.tile_pool(name="obank", bufs=4, space="PSUM") as obank,
        ):
            clms_sb = big.tile([LSIZE, T], F32)
            st_sb = big.tile([LSIZE, ST_COLS], F32)
            a1_sb = big.tile([128, ST_COLS], F32)
            nc.sync.dma_start(clms_sb[:], d_clms[:])
            nc.sync.dma_start(st_sb[:], d_st[:])
            nc.sync.dma_start(a1_sb[:], d_a1[:])

            w_tiles = [None] * CHUNKS

            def stage1(c):
                wps = wps_pool.tile([128, CHUNK_COLS], F32)
                nc.tensor.matmul(
                    wps[:],
                    st_sb[:, CHUNK_COLS * c : CHUNK_COLS * (c + 1)],
                    clms_sb[:],
                    start=True, stop=True,
                )
                wt = wpool.tile([128, CHUNK_COLS], F32)
                if c % 2 == 0:
                    nc.vector.tensor_copy(wt[:], wps[:])
                else:
                    nc.scalar.activation(wt[:], wps[:],
                                         mybir.ActivationFunctionType.Copy)
                w_tiles[c] = wt

            def stage2(g, o_tile):
                banks = [obank.tile([128, 512], F32, tag="obank", name=f"bank_{g}_{b}")
                         for b in range(BANDS)]
                # interleave across bands so different row-groups can overlap
                for cc in range(GROUP_CHUNKS):
                    c = GROUP_CHUNKS * g + cc
                    for b in range(BANDS):
                        p = SLOT_P * b
                        nc.tensor.matmul(
                            banks[b][:, 128 * cc : 128 * (cc + 1)],
                            a1_sb[p : p + SLOT_P,
                                  CHUNK_COLS * c : CHUNK_COLS * (c + 1)],
                            w_tiles[c][p : p + SLOT_P, :],
                            start=True, stop=True,
                            tile_position=(p, 0),
                        )
                for b in range(BANDS):
                    dst = o_tile[:, 512 * b : 512 * (b + 1)]
                    if b % 2 == 0:
                        nc.vector.tensor_copy(dst, banks[b][:])
                    else:
                        nc.scalar.activation(dst, banks[b][:],
                                             mybir.ActivationFunctionType.Copy)

            # software pipeline: stage-1 one group ahead of stage-2
            for c in range(GROUP_CHUNKS):
                stage1(c)
            for g in range(GROUPS):
                if g + 1 < GROUPS:
                    for c in range(GROUP_CHUNKS * (g + 1),
                                   GROUP_CHUNKS * (g + 2)):
                        stage1(c)
                o_tile = stage.tile([128, 2048], F32)
                stage2(g, o_tile)
                nc.sync.dma_start(d_out[:, 2048 * g : 2048 * (g + 1)], o_tile[:])
    nc.compile()
    return nc


def _install_profile_hook():
    """Register the NTFF profile hook (used only when BASS_TRACE=1)."""
    try:
        import antenv
        from concourse import bass_utils
        if "antenv.axon_hooks" not in sys.modules:
            mod = types.ModuleType("antenv.axon_hooks")
            mod._hook = None
            mod.set_axon_ntff_profile_hook = lambda h: setattr(mod, "_hook", h)
            mod.get_axon_ntff_profile_hook = lambda: mod._hook
            sys.modules["antenv.axon_hooks"] = mod
            antenv.axon_hooks = mod
        from trn_agent_boot.trn_boot import _ntff_profile_via_ctypes
        sys.modules["antenv.axon_hooks"].set_axon_ntff_profile_hook(
            _ntff_profile_via_ctypes("/opt/axon/libaxon_pjrt.so"))
        bass_utils.upload_artifacts = lambda tmpdir: f"local:{tmpdir}"
    except Exception as e:  # profiling is best-effort; execution must not break
        print(f"kernel: profile hook unavailable ({e})", file=sys.stderr)


LAST_EXEC_TIME_NS = None


def _build_core_inputs(clms, C, core):
    """s_t [64, ST_COLS] and a1s [128, ST_COLS] for one core."""
    s_t = np.zeros((LSIZE, ST_COLS), np.float32)
    a1s = np.zeros((128, ST_COLS), np.float32)
    base = SLOTS * core
    for s in range(SLOTS):
        pi, k = SLOT_TABLE[base + s]
        l1, l2, lo, hi, rows, po = PAIRS[pi]
        d1, d2 = 2 * l1 + 1, 2 * l2 + 1
        c, b = divmod(s, BANDS)
        col = CHUNK_COLS * c + SLOT_P * b
        # S^T column (col+i) rows l2^2..: C[k, l1^2+i, l2^2+j]
        blk = C[k, l1 * l1 : l1 * l1 + d1, l2 * l2 : l2 * l2 + d2]  # [d1, d2]
        s_t[l2 * l2 : l2 * l2 + d2, col : col + d1] = blk.T
        a1s[SLOT_P * b : SLOT_P * b + d1, CHUNK_COLS * c : CHUNK_COLS * (c + 1)] = \
            clms[l1 * l1 : l1 * l1 + d1, :]
    return s_t, a1s


def kernel(clms, C):
    global _NC, LAST_EXEC_TIME_NS
    from concourse.bass_utils import run_bass_kernel_spmd

    trace = os.environ.get("BASS_TRACE", "0") == "1"
    if trace:
        _install_profile_hook()

    clms = np.ascontiguousarray(np.asarray(clms, dtype=np.float32))
    C = np.ascontiguousarray(np.asarray(C, dtype=np.float32))

    if _NC is None:
        _NC = _build_nc()

    in_maps = []
    for core in range(NCORES):
        s_t, a1s = _build_core_inputs(clms, C, core)
        in_maps.append({"clms": clms, "s_t": s_t, "a1s": a1s})

    res = run_bass_kernel_spmd(_NC, in_maps, list(range(NCORES)), trace=trace)
    LAST_EXEC_TIME_NS = res.exec_time_ns

    # ---------------- host reassembly ----------------
    # G[global_row] = [T, T] matrix (t1, t2)
    G = np.empty((TOTAL_ROWS, T, T), np.float32)
    for core in range(NCORES):
        o = res.results[core]["o"]          # [128, OUT_COLS]
        for s in range(SLOTS):
            cb = _slot_colbase(s)
            G[SLOTS * core + s] = o[:, cb : cb + T]
    G = G.reshape(TOTAL_ROWS, T * T)

    out = []
    for l in range(L + 1):
        blocks = []
        for (l1, l2, lo, hi, rows, po) in PAIRS:
            if lo <= l <= hi:
                r0 = po + (l * l - lo * lo)
                blocks.append(G[r0 : r0 + 2 * l + 1, :])
        out.append(np.concatenate(blocks, axis=1))
    return tuple(out)


# revision 4
# speedup vs baseline: 1.8146x; 1.8146x over previous
"""Trainium2 Bass kernel for nn_CgTransform (L=7, T=128, 8 NeuronCores).

Math: for each (l1,l2) block pair and each kept output row k=(l,m):
    OUT_k[t1,t2] = sum_{i,j} C[k, l1^2+i, l2^2+j] * clms[l1^2+i, t1] * clms[l2^2+j, t2]
                 = A1^T (C_k A2).

Symmetry (verified numerically): CG coefficient symmetry gives
    OUT_{(l2,l1),k} = (-1)^{l1+l2-l} * OUT_{(l1,l2),k}^T,
so the device computes only the 36 pairs with l1<=l2 (1378 of 2416 output
rows); the host mirrors the rest by transposing [T,T] blocks.

Device restructuring (per output row k = one "slot"):
  stage 1:  W_k[i, t2] = sum_j C_k[i,j] * clms[l2^2+j, t2]
            -> dense matmul W_chunk = S_chunk^T.T @ clms per 128-partition
               chunk; 4 slots per chunk at 32-partition bands.
  stage 2:  OUT_k[t1,t2] = sum_i A1pad[i,t1] * W_k[i,t2]
            -> slots are grouped into CELLS of 4 consecutive k of the SAME
               pair spread over the 4 chunks of a group, so one N=512 matmul
               (lhsT = padded A1 at band 32b, rhs = the group's W band rows)
               computes 4 slots. tile_position=(32b, 0); each cell gets its
               own PSUM bank (HW: different tile_positions must not share a
               bank).

Sharding: 384 padded cells split 48 per core (identical SPMD program,
per-core S/A1 data). Output layout per core: [t1, group, band, j, t2]; host
transposes to [k, t1, t2] and reassembles the 8-tuple.

Env KERNEL_F32R=1: stage-2 runs in float32r (~4x faster matmul streaming,
~1e-4 rel err instead of ~2e-7).
"""
import os
import sys
import types

if "/opt/trn_rl_repo" not in sys.path:
    sys.path.insert(0, "/opt/trn_rl_repo")

import numpy as np

L = 7
LSIZE = (L + 1) ** 2          # 64
T = 128
NCORES = 8
SLOT_P = 32                   # partitions per slot band
BANDS = 4                     # bands (cells) per group
GROUP_CHUNKS = 4              # chunks per group == slots per cell

# ------------------------------------------------------------ pair tables
PAIRS = []                    # full table: (l1, l2, lo, hi, rows, row_offset)
_off = 0
for _l1 in range(L + 1):
    for _l2 in range(L + 1):
        _lo, _hi = abs(_l1 - _l2), min(_l1 + _l2, L)
        _rows = (_hi + 1) ** 2 - _lo * _lo
        PAIRS.append((_l1, _l2, _lo, _hi, _rows, _off))
        _off += _rows
TOTAL_ROWS = _off             # 2416

UP_PAIRS = [i for i, p in enumerate(PAIRS) if p[0] <= p[1]]   # 36 pair idxs

# global cell list: (full_pair_idx, k_base); one pair per cell, 4 k's
CELL_TABLE = []
for _pi in UP_PAIRS:
    _l1, _l2, _lo, _hi, _rows, _po = PAIRS[_pi]
    for _kb in range(_lo * _lo, (_hi + 1) ** 2, GROUP_CHUNKS):
        CELL_TABLE.append((_pi, _kb))
N_CELLS = len(CELL_TABLE)     # 356

CELLS_PER_CORE = -(-(-(-N_CELLS // NCORES)) // BANDS) * BANDS  # 48
GROUPS = CELLS_PER_CORE // BANDS       # 12
CHUNKS = GROUPS * GROUP_CHUNKS         # 48
SLOTS = CELLS_PER_CORE * GROUP_CHUNKS  # 192 per core
ST_COLS = CHUNKS * T                   # 6144
A1_COLS = GROUPS * T                   # 1536
OUT_COLS = SLOTS * T                   # 24576

_NC = None
LAST_EXEC_TIME_NS = None


def _build_nc(use_f32r):
    import concourse.bacc as bacc
    import concourse.mybir as mybir
    import concourse.tile as tile

    F32 = mybir.dt.float32
    WDT = mybir.dt.float32r if use_f32r else F32
    nc = bacc.Bacc("TRN2", target_bir_lowering=False, debug=False, num_devices=1)
    d_clms = nc.dram_tensor("clms", [LSIZE, T], F32, kind="ExternalInput")
    d_st = nc.dram_tensor("s_t", [LSIZE, ST_COLS], F32, kind="ExternalInput")
    d_a1 = nc.dram_tensor("a1s", [128, A1_COLS], WDT, kind="ExternalInput")
    d_out = nc.dram_tensor("o", [128, OUT_COLS], F32, kind="ExternalOutput")

    Copy = mybir.ActivationFunctionType.Copy

    with tile.TileContext(nc) as tc:
        with (
            tc.tile_pool(name="big", bufs=1) as big,
            tc.tile_pool(name="wpool", bufs=3) as wpool,
            tc.tile_pool(name="stage", bufs=3) as stage,
            tc.tile_pool(name="wps", bufs=2, space="PSUM") as wps_pool,
            tc.tile_pool(name="obank", bufs=4, space="PSUM") as obank,
        ):
            clms_sb = big.tile([LSIZE, T], F32)
            st_sb = big.tile([LSIZE, ST_COLS], F32)
            a1_sb = big.tile([128, A1_COLS], WDT)
            nc.sync.dma_start(clms_sb[:], d_clms[:])
            nc.sync.dma_start(st_sb[:], d_st[:])
            nc.sync.dma_start(a1_sb[:], d_a1[:])

            w_tiles = [None] * GROUPS

            def stage1(g):
                wt = wpool.tile([128, GROUP_CHUNKS * T], WDT, name=f"w_{g}",
                                tag="w")
                for j in range(GROUP_CHUNKS):
                    c = GROUP_CHUNKS * g + j
                    wps = wps_pool.tile([128, T], F32, name=f"wps_{c}", tag="wps")
                    nc.tensor.matmul(
                        wps[:],
                        st_sb[:, T * c : T * (c + 1)],
                        clms_sb[:],
                        start=True, stop=True,
                    )
                    dst = wt[:, T * j : T * (j + 1)]
                    if use_f32r or j % 2 == 0:
                        nc.vector.tensor_copy(dst, wps[:])
                    else:
                        nc.scalar.activation(dst, wps[:], Copy)
                w_tiles[g] = wt

            def stage2(g, o_tile):
                for b in range(BANDS):
                    bank = obank.tile([128, 512], F32, name=f"bank_{g}_{b}",
                                      tag="obank")
                    p = SLOT_P * b
                    nc.tensor.matmul(
                        bank[:],
                        a1_sb[p : p + SLOT_P, T * g : T * (g + 1)],
                        w_tiles[g][p : p + SLOT_P, :],
                        start=True, stop=True,
                        tile_position=(p, 0),
                    )
                    dst = o_tile[:, 512 * b : 512 * (b + 1)]
                    if (b + (0 if use_f32r else g)) % 4 < 2:
                        nc.vector.tensor_copy(dst, bank[:])
                    else:
                        nc.scalar.activation(dst, bank[:], Copy)

            stage1(0)
            for g in range(GROUPS):
                if g + 1 < GROUPS:
                    stage1(g + 1)
                o_tile = stage.tile([128, 2048], F32, name=f"o_{g}", tag="o")
                stage2(g, o_tile)
                nc.sync.dma_start(d_out[:, 2048 * g : 2048 * (g + 1)], o_tile[:])
    nc.compile()
    return nc


def _install_profile_hook():
    try:
        import antenv
        from concourse import bass_utils
        if "antenv.axon_hooks" not in sys.modules:
            mod = types.ModuleType("antenv.axon_hooks")
            mod._hook = None
            mod.set_axon_ntff_profile_hook = lambda h: setattr(mod, "_hook", h)
            mod.get_axon_ntff_profile_hook = lambda: mod._hook
            sys.modules["antenv.axon_hooks"] = mod
            antenv.axon_hooks = mod
        from trn_agent_boot.trn_boot import _ntff_profile_via_ctypes
        sys.modules["antenv.axon_hooks"].set_axon_ntff_profile_hook(
            _ntff_profile_via_ctypes("/opt/axon/libaxon_pjrt.so"))
        bass_utils.upload_artifacts = lambda tmpdir: f"local:{tmpdir}"
    except Exception as e:
        print(f"kernel: profile hook unavailable ({e})", file=sys.stderr)


def _build_core_inputs(clms, C, core):
    s_t = np.zeros((LSIZE, ST_COLS), np.float32)
    a1s = np.zeros((128, A1_COLS), np.float32)
    base = CELLS_PER_CORE * core
    for c_loc in range(CELLS_PER_CORE):
        ci = base + c_loc
        if ci >= N_CELLS:
            break
        pi, kb = CELL_TABLE[ci]
        l1, l2, lo, hi, rows, po = PAIRS[pi]
        d1, d2 = 2 * l1 + 1, 2 * l2 + 1
        g, b = divmod(c_loc, BANDS)
        a1s[SLOT_P * b : SLOT_P * b + d1, T * g : T * (g + 1)] = \
            clms[l1 * l1 : l1 * l1 + d1, :]
        kmax = (hi + 1) ** 2
        for j in range(GROUP_CHUNKS):
            k = kb + j
            if k >= kmax:
                break
            col = T * (GROUP_CHUNKS * g + j) + SLOT_P * b
            blk = C[k, l1 * l1 : l1 * l1 + d1, l2 * l2 : l2 * l2 + d2]
            s_t[l2 * l2 : l2 * l2 + d2, col : col + d1] = blk.T
    return s_t, a1s


def kernel(clms, C):
    global _NC, LAST_EXEC_TIME_NS
    from concourse.bass_utils import run_bass_kernel_spmd

    trace = os.environ.get("BASS_TRACE", "0") == "1"
    use_f32r = os.environ.get("KERNEL_F32R", "0") == "1"
    if trace:
        _install_profile_hook()

    clms = np.ascontiguousarray(np.asarray(clms, dtype=np.float32))
    C = np.ascontiguousarray(np.asarray(C, dtype=np.float32))

    if _NC is None:
        _NC = _build_nc(use_f32r)

    in_maps = []
    for core in range(NCORES):
        s_t, a1s = _build_core_inputs(clms, C, core)
        in_maps.append({"clms": clms, "s_t": s_t, "a1s": a1s})

    res = run_bass_kernel_spmd(_NC, in_maps, list(range(NCORES)), trace=trace)
    LAST_EXEC_TIME_NS = res.exec_time_ns

    # ---------------- host reassembly ----------------
    G = np.empty((TOTAL_ROWS, T, T), np.float32)
    for core in range(NCORES):
        o = res.results[core]["o"]          # [128, OUT_COLS]
        base = CELLS_PER_CORE * core
        for c_loc in range(CELLS_PER_CORE):
            ci = base + c_loc
            if ci >= N_CELLS:
                break
            pi, kb = CELL_TABLE[ci]
            l1, l2, lo, hi, rows, po = PAIRS[pi]
            kmax = (hi + 1) ** 2
            g, b = divmod(c_loc, BANDS)
            for j in range(GROUP_CHUNKS):
                k = kb + j
                if k >= kmax:
                    break
                cb = 2048 * g + 512 * b + 128 * j
                G[po + (k - lo * lo)] = o[:, cb : cb + T]

    # mirror lower pairs (l1 > l2) from upper: OUT = sign * OUT_upper^T
    ls = np.arange(LSIZE)
    l_of_k = np.floor(np.sqrt(ls)).astype(np.int64)
    for pi, (l1, l2, lo, hi, rows, po) in enumerate(PAIRS):
        if l1 <= l2:
            continue
        up = PAIRS[8 * l2 + l1]
        po_u = up[5]
        ks = np.arange(lo * lo, (hi + 1) ** 2)
        sign = ((-1.0) ** (l1 + l2 - l_of_k[ks])).astype(np.float32)
        G[po : po + rows] = sign[:, None, None] * \
            G[po_u : po_u + rows].transpose(0, 2, 1)

    Gf = G.reshape(TOTAL_ROWS, T * T)
    out = []
    for l in range(L + 1):
        blocks = []
        for (l1, l2, lo, hi, rows, po) in PAIRS:
            if lo <= l <= hi:
                r0 = po + (l * l - lo * lo)
                blocks.append(Gf[r0 : r0 + 2 * l + 1, :])
        out.append(np.concatenate(blocks, axis=1))
    return tuple(out)
